# revision 1
# baseline (speedup 1.0000x reference)
"""Dynamic-weight conv2d (DYDConv2d) Trainium2 kernel.

Problem: per-sample SE-gated mixture of K=4 conv filter banks, then a 3x3
conv (pad 1) with the per-sample aggregated weights.

  pooled = mean_hw(x)                     [B, C]
  h      = relu(pooled @ fc1_w.T)         [B, 65]
  y      = h @ fc2_w.T + fc2_b            [B, 1024]
  prob   = softmax(y.reshape(B,4,256)/30) [B, 4, 256]
  agg    = einsum('bko,kof->bof', prob, W.reshape(4,256,2304))
  out[b] = conv2d(x[b], agg[b].reshape(256,256,3,3), pad=1)

Sharding: pure data-parallel over batch. 8 cores x 2 samples each; every
core holds the full filter bank + SE params. No cross-core comm.

Per-core plan (all conv matmuls bf16, f32 accumulation in PSUM):
 - x loaded f32, cast to a zero-padded bf16 [128, 66, 68] layout per
   ci-block; the cast op also emits the pooled sum (free accum_out).
 - SE chain runs in "transposed" layout so prob lands as per-partition
   scalars: psum_y [128, 8] columns map to (k, o_blk).
 - agg[o, (ci,off)] built on DVE with 1 tensor_scalar + 3 fused
   scalar_tensor_tensor ops per o-block from the pre-cast bf16 W.
 - aggT[ci, off, o] produced by 36 PE transposes (128x128 blocks), copied
   psum->sbuf in batches.
 - conv = 9 shifted matmuls per ci-block accumulating over (ci_blk, off)
   into psum [128, 512] banks; psum->sbuf copy; DMA to HBM.

Emission order is tuned so the serial DMA resource streams
x(s0) -> W(o-blk 0) -> W(o-blk 1) -> x(s1) -> outputs, and the conv for
sample 0 / o-block 0 starts as soon as the first half of W has landed.
"""
import sys

for _p in ("/opt/trn_rl_repo", "/root/.axon_site/_ro/trn_rl_repo"):
    if _p not in sys.path:
        sys.path.insert(0, _p)

import numpy as np

try:  # persistent jax compile cache: makes repeat invocations fast
    import jax
    jax.config.update("jax_compilation_cache_dir", "/tmp/jaxcache")
except Exception:
    pass

import concourse.bass as bass
import concourse.tile as tile
from concourse import bacc, mybir
from concourse.bass_utils import run_bass_kernel_spmd
from concourse.masks import make_identity

F32 = mybir.dt.float32
BF16 = mybir.dt.bfloat16
MULT = mybir.AluOpType.mult
ADD = mybir.AluOpType.add
ACT_COPY = mybir.ActivationFunctionType.Copy
ACT_RELU = mybir.ActivationFunctionType.Relu
ACT_EXP = mybir.ActivationFunctionType.Exp

B, C, H, W = 16, 256, 64, 64
O, K, HID = 256, 4, 65
KK = 3  # kernel spatial size
NOFF = KK * KK  # 9
CF = C * NOFF  # 2304  (ci, off) flattened
N_CORES = 8
BS = B // N_CORES  # samples per core
TEMP = 30.0
# padded x layout: row stride 68 (left pad 2 keeps 4B alignment), 66 rows
PH, PW = H + 2, 68
HWCHUNKS = (1536, 1536, 512, 512)  # free-dim chunking of the 4096 out pixels
TGROUPS = ((0, 4), (4, 8), (8, 9))  # transpose off-batches


def build_kernel(stage=4):
    """stage: 1=through agg, 2=+transposes, 3=+1 conv chunk, 4=full."""
    nc = bacc.Bacc("TRN2", target_bir_lowering=False, debug=False,
                   num_devices=N_CORES)
    x_d = nc.dram_tensor("x", [BS, C, H, W], F32, kind="ExternalInput")
    fc1_d = nc.dram_tensor("fc1_w", [HID, C], F32, kind="ExternalInput")
    fc2_d = nc.dram_tensor("fc2_w", [K * O, HID], F32, kind="ExternalInput")
    fc2b_d = nc.dram_tensor("fc2_b", [K * O], F32, kind="ExternalInput")
    w_d = nc.dram_tensor("weight", [K, O, C, KK, KK], F32, kind="ExternalInput")
    out_d = nc.dram_tensor("out", [BS, O, H, W], F32, kind="ExternalOutput")
    dbg_d = None
    if stage < 3:
        dbg_d = nc.dram_tensor("dbg", [BS, 2, 128, CF], BF16,
                               kind="ExternalOutput")

    with tile.TileContext(nc) as tc:
        _body(nc, tc, x_d, fc1_d, fc2_d, fc2b_d, w_d, out_d, stage, dbg_d)
    nc.compile()
    return nc


def _body(nc, tc, x_d, fc1_d, fc2_d, fc2b_d, w_d, out_d, stage=4, dbg_d=None):
    with (
        tc.tile_pool(name="const", bufs=1) as constp,
        tc.tile_pool(name="wbank", bufs=1) as wbank,
        tc.tile_pool(name="wstage", bufs=5) as wstage,
        tc.tile_pool(name="xf", bufs=2) as xfp,
        tc.tile_pool(name="xb", bufs=1) as xbp,
        tc.tile_pool(name="aggp", bufs=2) as aggp,
        tc.tile_pool(name="aggtp", bufs=2) as aggtp,
        tc.tile_pool(name="small", bufs=2) as smallp,
        tc.tile_pool(name="ost", bufs=3) as ostp,
        tc.tile_pool(name="psc", bufs=2, space=bass.MemorySpace.PSUM) as pscp,
        tc.tile_pool(name="pst", bufs=2, space=bass.MemorySpace.PSUM) as pstp,
    ):
        # ---- params + halo init -----------------------------------------
        # fc1/fc2 are loaded in their natural (contiguous) layouts and
        # transposed on-chip — element-strided gather DMAs are descriptor-
        # bound (~30us for fc2) and would hog the DMA engines at startup.
        with nc.named_scope("params"):
            ident = constp.tile([128, 128], BF16)
            make_identity(nc, ident[:])
            ident32 = constp.tile([128, 128], F32)
            make_identity(nc, ident32[:])
            fc1n = constp.tile([128, C], F32)  # rows 0..64 = fc1_w
            nc.sync.dma_start(fc1n[0:HID, :], fc1_d[:])
            fc2n = constp.tile([128, 8, HID], F32)  # [i_in_blk, i_blk, j]
            nc.sync.dma_start(
                fc2n[:], bass.AP(fc2_d, 0, [[HID, 128], [128 * HID, 8],
                                            [1, HID]]))
            fc1t = constp.tile([128, 2, HID], F32)  # [ci_in_blk, ci_blk, j]
            for blk in range(2):
                tps = pstp.tile([128, HID], F32, tag="pt", name=f"tp1_{blk}")
                nc.tensor.transpose(tps[:], fc1n[0:HID, blk * 128:(blk + 1) * 128],
                                    ident32[0:HID, 0:HID])
                nc.scalar.copy(fc1t[:, blk, :], tps[:])
            fc2t = constp.tile([128, K * O], F32)  # unused rows 66..127
            # rows 0..64 = fc2_w.T ; row 65 = fc2_b (bias folded into matmul)
            for half in range(2):
                tps = pstp.tile([128, 512], F32, tag="pt", name=f"tp2_{half}")
                for c in range(4):
                    nc.tensor.transpose(tps[0:HID, c * 128:(c + 1) * 128],
                                        fc2n[:, half * 4 + c, :], ident32[:])
                nc.vector.tensor_copy(fc2t[0:HID, half * 512:(half + 1) * 512],
                                      tps[0:HID, :])
            nc.sync.dma_start(fc2t[HID:HID + 1, :], fc2b_d[:].unsqueeze(0))
            # zero only the halo cells (full-tile memsets cost ~7.6us each)
            xb = [xbp.tile([128, 2, PH, PW], BF16, name=f"xb{s}")
                  for s in range(BS)]
            for s in range(BS):
                for blk in range(2):
                    nc.gpsimd.memset(xb[s][:, blk, 0, :], 0.0)
                    nc.gpsimd.memset(xb[s][:, blk, PH - 1, :], 0.0)
                    nc.gpsimd.memset(xb[s][:, blk, 0:PH - 1, PW - 2:PW], 0.0)
                    nc.gpsimd.memset(xb[s][:, blk, 1:PH, 0:2], 0.0)

        # ---- x loads + cast/pool ----------------------------------------
        # s0 casts on ACT/DVE (fast, feed the s0 SE chain); s1 casts on the
        # otherwise-idle GPSIMD so they don't block DVE's W-cast/agg chain.
        pooled, se = [], []
        # (ci_blk, pooled col) pairs for the z accumulation, per sample
        zcols = [[(0, 0), (0, 1), (1, 2), (1, 3)], [(0, 0), (1, 1)]]

        def xload_blk(s, blk):
            """s0: two 32-row quarter DMAs + DVE casts (shortens the pooled
            critical path); s1: one full-block DMA + gpsimd cast."""
            with nc.named_scope(f"xload{s}"):
                if blk == 0:
                    pooled.append(smallp.tile([128, 4], F32, tag="pooled",
                                              name=f"pooled{s}"))
                if s > 0:
                    xf = xfp.tile([128, H, W], F32, tag="xf",
                                  name=f"xf{s}_{blk}")
                    nc.sync.dma_start(xf[:], x_d[s, blk * 128:(blk + 1) * 128])
                    interior = xb[s][:, blk, 1:H + 1, 2:W + 2]
                    if blk == 0:
                        nc.scalar.activation(interior, xf[:], ACT_COPY,
                                             accum_out=pooled[s][:, 0:1])
                    else:
                        nc.vector.tensor_scalar(interior, xf[:], 1.0, None,
                                                MULT, ADD,
                                                accum_out=pooled[s][:, 1:2])
                    return
                for hh in range(2):
                    xq = xfp.tile([128, H // 2, W], F32, tag="xq",
                                  name=f"xq{s}_{blk}_{hh}")
                    nc.sync.dma_start(
                        xq[:], x_d[s, blk * 128:(blk + 1) * 128,
                                   hh * 32:(hh + 1) * 32])
                    interior = xb[s][:, blk, 1 + 32 * hh:33 + 32 * hh, 2:W + 2]
                    nc.vector.tensor_scalar(
                        interior, xq[:], 1.0, None, MULT, ADD,
                        accum_out=pooled[s][:, 2 * blk + hh:2 * blk + hh + 1])

        def se_chain(s):
            with nc.named_scope(f"se{s}"):
                z_ps = pstp.tile([128, 1], F32, tag="pt", name=f"z{s}")
                cols = zcols[s]
                for i, (blk, col) in enumerate(cols):
                    nc.tensor.matmul(z_ps[0:HID, :], fc1t[:, blk, :],
                                     pooled[s][:, col:col + 1],
                                     start=(i == 0), stop=(i == len(cols) - 1))
                h_ext = smallp.tile([128, 1], F32, tag="hext", name=f"hext{s}")
                nc.vector.memset(h_ext[:], 1.0)  # row 65 stays 1.0 (bias row)
                # relu(z/4096): mean folded via scale (relu is scale-invariant)
                nc.scalar.activation(h_ext[0:HID, :], z_ps[0:HID, :], ACT_RELU,
                                     scale=1.0 / (H * W))
                y_ps = pstp.tile([128, K * 2], F32, tag="pt", name=f"y{s}")
                for c in range(K * 2):
                    nc.tensor.matmul(y_ps[:, c:c + 1],
                                     fc2t[0:HID + 1, c * 128:(c + 1) * 128],
                                     h_ext[0:HID + 1, :], start=True, stop=True)
                e = smallp.tile([128, K, 2], F32, tag="e", name=f"e{s}")
                nc.scalar.activation(e[:].rearrange("p a b -> p (a b)"),
                                     y_ps[:], ACT_EXP, scale=1.0 / TEMP)
                ssum = smallp.tile([128, 2], F32, tag="ssum", name=f"ssum{s}")
                er = e[:].rearrange("p k o -> p o k")
                nc.vector.tensor_reduce(ssum[:], er, mybir.AxisListType.X, ADD)
                rinv = smallp.tile([128, 2], F32, tag="rinv", name=f"rinv{s}")
                nc.vector.reciprocal(rinv[:], ssum[:])
                prob = smallp.tile([128, 2, K], F32, tag="prob", name=f"prob{s}")
                for ob in range(2):
                    nc.vector.tensor_scalar_mul(prob[:, ob], er[:, ob],
                                                rinv[:, ob:ob + 1])
                return prob

        # DMA queue order: x0, W(ob0), W(ob1), x1, outs
        wb = [wbank.tile([128, K, C, NOFF], BF16, name=f"wb{ob}")
              for ob in range(2)]
        xload_blk(0, 0)
        xload_blk(0, 1)

        def load_w(ob):
            # ci-half-major chunks so agg/transposes for ci-block 0 can
            # start while ci-block 1 is still in flight on the DMA ring
            with nc.named_scope(f"wload{ob}"):
                for cb in range(2):
                    for k in range(K):
                        wst = wstage.tile([128, CF // 2], F32, tag="wst")
                        nc.sync.dma_start(
                            wst[:],
                            w_d[k, ob * 128:(ob + 1) * 128,
                                cb * 128:(cb + 1) * 128].rearrange(
                                    "p c a b -> p (c a b)"))
                        dst = wb[ob][:, k, cb * 128:(cb + 1) * 128, :].rearrange(
                            "p c o -> p (c o)")
                        # all W casts on ACT: DVE owns the x casts + agg
                        # chain at startup and must not self-block
                        nc.scalar.copy(dst, wst[:])

        # agg + transposes for (s, ob), per ci-half -> ob-half of aggt tiles
        def agg_ob(s, ob, agg, aggt):
            for cb in range(2):
                asl = agg[ob][:, cb * 128:(cb + 1) * 128, :]
                with nc.named_scope(f"agg{s}_{ob}"):
                    nc.vector.tensor_scalar_mul(
                        asl, wb[ob][:, 0, cb * 128:(cb + 1) * 128, :],
                        se[s][:, ob, 0:1])
                    for k in range(1, K):
                        nc.vector.scalar_tensor_tensor(
                            asl, wb[ob][:, k, cb * 128:(cb + 1) * 128, :],
                            se[s][:, ob, k:k + 1], asl, MULT, ADD)
                if aggt is None:
                    continue
                with nc.named_scope(f"transp{s}_{ob}"):
                    for gi, (o0, o1) in enumerate(TGROUPS):
                        n = o1 - o0
                        pt = pstp.tile([128, 4, 128], BF16, tag="pt",
                                       name=f"pt{s}_{ob}_{cb}_{gi}")
                        for oi in range(n):
                            nc.tensor.transpose(
                                pt[:, oi, :],
                                agg[ob][:, cb * 128:(cb + 1) * 128, o0 + oi],
                                ident[:])
                        src = pt[:, 0:n, :]
                        dst = aggt[cb][:, o0:o1, ob * 128:(ob + 1) * 128]
                        if (cb * 3 + gi) % 2 == 0:
                            nc.scalar.copy(dst, src)
                        else:
                            nc.vector.tensor_copy(dst, src)

        def conv(s, aggt):
            out_hw = out_d[s].rearrange("o a b -> o (a b)")
            with nc.named_scope(f"conv{s}"):
                for ob in range(2 if stage >= 4 else 1):
                    c0 = 0
                    chunks = HWCHUNKS if stage >= 4 else HWCHUNKS[:1]
                    for ci, csz in enumerate(chunks):
                        pc = pscp.tile([128, max(HWCHUNKS)], F32, tag="conv",
                                       name=f"conv{s}_{ob}_{ci}")
                        for cb in range(2):
                            for off in range(NOFF):
                                dh, dw = off // KK - 1, off % KK - 1
                                lhsT = aggt[cb][:, off, ob * 128:(ob + 1) * 128]
                                for sub in range(csz // 512):
                                    h0 = (c0 + sub * 512) // W
                                    rhs = xb[s][:, cb, h0 + 1 + dh:h0 + 9 + dh,
                                                2 + dw:2 + dw + W]
                                    nc.tensor.matmul(
                                        pc[:, sub * 512:(sub + 1) * 512],
                                        lhsT, rhs,
                                        start=(cb == 0 and off == 0),
                                        stop=(cb == 1 and off == NOFF - 1))
                        ost = ostp.tile([128, max(HWCHUNKS)], F32, tag="ost")
                        if (ob * 3 + ci) % 2 == 0:
                            nc.scalar.copy(ost[:, 0:csz], pc[:, 0:csz])
                        else:
                            nc.vector.tensor_copy(ost[:, 0:csz], pc[:, 0:csz])
                        nc.sync.dma_start(
                            out_hw[ob * 128:(ob + 1) * 128, c0:c0 + csz],
                            ost[:, 0:csz])
                        c0 += csz

        def dbg_dump(s, tiles):
            for i in range(2):
                nc.sync.dma_start(dbg_d[s, i],
                                  tiles[i][:].rearrange("p a b -> p (a b)"))

        # sample 0: interleave with W arrival (ob 0 first)
        agg0 = [aggp.tile([128, C, NOFF], BF16, tag="agg", name=f"agg0_{ob}")
                for ob in range(2)]
        aggt0 = None
        if stage >= 2:
            aggt0 = [aggtp.tile([128, NOFF, O], BF16, tag="aggt",
                                name=f"aggt0_{cb}") for cb in range(2)]
        se.append(se_chain(0))
        load_w(0)
        agg_ob(0, 0, agg0, aggt0)
        load_w(1)
        agg_ob(0, 1, agg0, aggt0)
        xload_blk(1, 0)
        xload_blk(1, 1)
        se.append(se_chain(1))
        if stage == 1:
            dbg_dump(0, agg0)
        elif stage == 2:
            dbg_dump(0, aggt0)
        else:
            conv(0, aggt0)

        # sample 1
        agg1 = [aggp.tile([128, C, NOFF], BF16, tag="agg", name=f"agg1_{ob}")
                for ob in range(2)]
        aggt1 = None
        if stage >= 2:
            aggt1 = [aggtp.tile([128, NOFF, O], BF16, tag="aggt",
                                name=f"aggt1_{cb}") for cb in range(2)]
        for ob in range(2):
            agg_ob(1, ob, agg1, aggt1)
        if stage == 1:
            dbg_dump(1, agg1)
        elif stage == 2:
            dbg_dump(1, aggt1)
        else:
            conv(1, aggt1)


_NC_CACHE = None


def _get_nc():
    global _NC_CACHE
    if _NC_CACHE is None:
        _NC_CACHE = build_kernel()
    return _NC_CACHE


def make_in_maps(x, fc1_w, fc2_w, fc2_b, weight):
    x = np.ascontiguousarray(x, dtype=np.float32)
    shared = {
        "fc1_w": np.ascontiguousarray(fc1_w, dtype=np.float32),
        "fc2_w": np.ascontiguousarray(fc2_w, dtype=np.float32),
        "fc2_b": np.ascontiguousarray(fc2_b, dtype=np.float32),
        "weight": np.ascontiguousarray(weight, dtype=np.float32),
    }
    return [{"x": x[c * BS:(c + 1) * BS], **shared} for c in range(N_CORES)]


def kernel(x, fc1_w, fc2_w, fc2_b, weight):
    import time
    nc = _get_nc()
    in_maps = make_in_maps(x, fc1_w, fc2_w, fc2_b, weight)
    res = None
    for attempt in range(3):
        try:
            res = run_bass_kernel_spmd(nc, in_maps,
                                       core_ids=list(range(N_CORES)))
            break
        except Exception:
            # transient device wedge (NRT_EXEC_UNIT_UNRECOVERABLE); the
            # axon terminal recovers after a short wait
            if attempt == 2:
                raise
            time.sleep(60 * (attempt + 1))
    return np.concatenate([res.results[c]["out"] for c in range(N_CORES)],
                          axis=0).astype(np.float32)

